# revision 7
# baseline (speedup 1.0000x reference)
"""Masked dot-product attention on 8 Trainium2 NeuronCores.

Strategy (per core): head-parallel sharding. B*H = 64 (batch, head) pairs are
split 8 per core; each core runs the full attention for its heads, two heads
("a pair") at a time so the K=64 QK^T matmuls co-run on PE row-group halves.

Per-pair pipeline (S=2048, DK=64), in "S-transposed" layout so the PV matmul
needs no transpose of the huge exp matrix:
  S_T[kj, qi] = K @ Q^T        (PE, bf16, psum [128 kj, 2*512 qi] strips,
                                two heads row-group packed -> co-run)
  E_T = exp(S_T / sqrt(dk))    (ACT, PSUM -> SBUF bf16; no max-shift: logits
                                are ~N(0,1) so exp can't overflow)
  E_T *= (maskT == 0)          (DVE scalar_tensor_tensor, one op, bf16 2x)
  O_T[dv, qi] += V1[kj]^T E_T  (PE accumulate over kj; V1 has a ones column
                                so row dv=64 accumulates the softmax denom)
  O = (O_T^T)[:, :64] * recip(O_T^T[:, 64])   (PE transpose + DVE)

ACT's exp (1 elem/lane/cycle, ScalarE-only) is the hard engine floor
(~257us); everything else is scheduled to hide under it:
  - all input transposes (Q/K and the mask) go through DRAM scratch + the
    DMA XBAR transpose on the sync queue, not the PE
  - the int32 [S, S] mask streams in qb-aligned quarter-strips in exactly
    consumption order, converted int32->bf16 by a gpsimd SWDGE casting DMA
    (no DVE/ACT cost), so the 16 MiB mask read spreads over all of hp0
  - emission is software-pipelined with skew 2: the QK/EXP stream runs two
    steps ahead of mask-mult/PV, so a late mask quarter can't stall ACT
    through the in-order PE queue
  - output normalization is spread over the next block's steps
"""

import math
from collections import defaultdict

import numpy as np

import concourse.bass as bass
import concourse.mybir as mybir
import concourse.tile as tile
from concourse import bacc
from concourse.masks import make_identity

F32 = mybir.dt.float32
BF16 = mybir.dt.bfloat16
I32 = mybir.dt.int32
AF = mybir.ActivationFunctionType
ALU = mybir.AluOpType

N_CORES = 8

# Mask conversion via gpsimd SWDGE casting DMA (int32 SBUF -> bf16 DRAM).
# Fallback (False): DVE tensor_scalar converts to (1-m) bf16 before the
# DRAM round trip, and the mask-mult is a plain tensor_mul.
MASK_VIA_CAST_DMA = True


def build_attention_nc(nheads: int, S: int, DK: int, scale: float) -> bass.Bass:
    nc = bacc.Bacc("TRN2", target_bir_lowering=False, debug=False,
                   num_devices=N_CORES)

    q_d = nc.dram_tensor("queries", [nheads, S, DK], F32, kind="ExternalInput")
    k_d = nc.dram_tensor("keys", [nheads, S, DK], F32, kind="ExternalInput")
    v_d = nc.dram_tensor("values", [nheads, S, DK], F32, kind="ExternalInput")
    m_d = nc.dram_tensor("mask", [S, S], I32, kind="ExternalInput")
    o_d = nc.dram_tensor("out", [nheads, S, DK], F32, kind="ExternalOutput")

    DV1 = DK + 1          # V plus a ones column for softmax denominators
    PVM = 128             # PV stationary padded to 128 cols (enables FWL)
    NKJ = S // 128        # kj strips
    QBLK = 512
    NQ = S // QBLK        # qi blocks per head
    OC = QBLK // 128      # 128-row output chunks per block
    CH = S // 128         # (p c) staging chunks
    MQC = QBLK // 128     # mask staging chunks per quarter (rows per part /4)
    NP = nheads // 2      # head pairs
    SPP = NQ * NKJ        # steps per pair (64)
    TOT = NP * SPP        # total steps (256)
    SKEW = 2              # QK/EXP emission runs this many steps ahead

    assert nheads % 2 == 0

    with tile.TileContext(nc) as tc:
        with (
            tc.tile_pool(name="consts", bufs=1) as consts,
            tc.tile_pool(name="maskT", bufs=1) as maskpool,
            tc.tile_pool(name="mst", bufs=6) as mstp,
            tc.tile_pool(name="stage", bufs=5) as stage,
            tc.tile_pool(name="qkT", bufs=2) as qkt,
            tc.tile_pool(name="vp", bufs=2) as vp,
            tc.tile_pool(name="ep", bufs=8) as ep,
            tc.tile_pool(name="outp", bufs=2) as outp,
            tc.tile_pool(name="small", bufs=2) as small,
            tc.tile_pool(name="spsum", bufs=2, space="PSUM") as spsum,
            tc.tile_pool(name="opsum", bufs=3, space="PSUM") as opsum,
            tc.tile_pool(name="tpsum", bufs=1, space="PSUM") as tpsum,
            tc.tile_pool(name="dram_scr", bufs=2, space="DRAM") as dram_scr,
        ):
            ident_f = consts.tile([DV1, DV1], F32)
            make_identity(nc, ident_f)

            maskT = [
                maskpool.tile([128, S], BF16, tag=f"maskT{kt}",
                              name=f"maskT_{kt}")
                for kt in range(NKJ)
            ]

            # state carried between schedule slots
            qT2 = [None, None]   # double-buffered by pair parity
            kT2 = [None, None]
            v1s = [[None, None], [None, None]]  # [parity][i]
            ps_o = {}            # qb-parity -> [ps_o_h0, ps_o_h1]
            e_ts = {}            # step -> e tile
            out_state = {}       # per-output-phase scratch

            sched = defaultdict(list)   # slot -> [fn]

            def at(slot, fn):
                sched[max(0, slot)].append(fn)

            # ---------------- per-pair q/k/v prefetch ----------------
            def qk_load(hp, name, src, i):
                def fn():
                    nat = stage.tile([128, CH, DK], F32, tag="nat",
                                     name=f"nat_{name}_{hp}_{i}")
                    nc.sync.dma_start(
                        out=nat,
                        in_=src[2 * hp + i].rearrange("(p c) d -> p c d",
                                                      p=128))
                    out_state[("nat", hp, name, i)] = nat
                return fn

            def qk_cast(hp, name, i):
                def fn():
                    nat = out_state.pop(("nat", hp, name, i))
                    key = ("natb", hp, name)
                    if key not in out_state:
                        out_state[key] = stage.tile(
                            [128, CH, 2 * DK], BF16, tag="natb", bufs=2,
                            name=f"natb_{name}_{hp}")
                    natb = out_state[key]
                    nc.vector.tensor_copy(
                        natb[:, :, i * DK:(i + 1) * DK], nat)
                return fn

            def qk_scr(hp, name):
                def fn():
                    natb = out_state.pop(("natb", hp, name))
                    scr = dram_scr.tile([S, 2 * DK], BF16, tag=f"{name}scr",
                                        name=f"scr_{name}_{hp}")
                    nc.sync.dma_start(
                        out=scr.rearrange("(p c) e -> p c e", p=128),
                        in_=natb)
                    out_state[("scr", hp, name)] = scr
                return fn

            def qk_transpose(hp, name):
                def fn():
                    scr = out_state.pop(("scr", hp, name))
                    tT = qkt.tile([128, S], BF16, tag=f"{name}T",
                                  name=f"{name}T_{hp}")
                    nc.sync.dma_start(out=tT, in_=scr, transpose=True)
                    (qT2 if name == "q" else kT2)[hp % 2] = tT
                return fn

            def v_load(hp, i):
                def fn():
                    vnat = stage.tile([128, CH, DK], F32, tag="vnat", bufs=3,
                                      name=f"vnat_{hp}_{i}")
                    nc.sync.dma_start(
                        out=vnat,
                        in_=v_d[2 * hp + i].rearrange("(c p) d -> p c d",
                                                      p=128))
                    out_state[("vnat", hp, i)] = vnat
                return fn

            def v_build(hp, i):
                def fn():
                    vnat = out_state.pop(("vnat", hp, i))
                    v1 = vp.tile([128, CH, PVM], BF16, tag=f"v1_{i}",
                                 name=f"v1_{2 * hp + i}")
                    nc.vector.tensor_copy(v1[:, :, 0:DK], vnat)
                    nc.gpsimd.memset(v1[:, :, DK:DV1], 1.0)
                    nc.gpsimd.memset(v1[:, :, DV1:PVM], 0.0)
                    v1s[hp % 2][i] = v1
                return fn

            for hp in range(NP):
                base = (hp - 1) * SPP + 8   # inside previous pair's steps
                for i in (0, 1):
                    at(base + i, qk_load(hp, "k", k_d, i))
                    at(base + 2 + i, qk_load(hp, "q", q_d, i))
                    at(base + 4 + i, qk_cast(hp, "k", i))
                    at(base + 6 + i, qk_cast(hp, "q", i))
                at(base + 8, qk_scr(hp, "k"))
                at(base + 9, qk_scr(hp, "q"))
                at(base + 10, qk_transpose(hp, "k"))
                at(base + 12, qk_transpose(hp, "q"))
                at(base + 14, v_load(hp, 0))
                at(base + 16, v_load(hp, 1))
                at(base + 22, v_build(hp, 0))
                at(base + 24, v_build(hp, 1))

            # ---------------- mask quarter pipeline ----------------
            # quarter Q = qtr*NKJ + kt is consumed by back-step
            # s_b = qtr*NKJ + kt (during pair 0), i.e. slot Q + SKEW.
            def m_load(kt, qtr):
                def fn():
                    mst = mstp.tile([128, MQC, 128], I32, tag="mraw",
                                    name=f"mraw_{kt}_{qtr}")
                    src = m_d[qtr * QBLK:(qtr + 1) * QBLK,
                              kt * 128:(kt + 1) * 128].rearrange(
                                  "(p c) j -> p c j", p=128)
                    nc.sync.dma_start(out=mst, in_=src)
                    out_state[("mst", kt, qtr)] = mst
                return fn

            def m_stage(kt, qtr):
                def fn():
                    mst = out_state.pop(("mst", kt, qtr))
                    mscr = dram_scr.tile([QBLK, 128], BF16, tag="mscr",
                                         bufs=5, name=f"mscr_{kt}_{qtr}")
                    dst = mscr.rearrange("(p c) j -> p c j", p=128)
                    if MASK_VIA_CAST_DMA:
                        # SWDGE casting DMA: int32 {0,1} -> bf16 {0.0,1.0}
                        nc.gpsimd.dma_start(out=dst, in_=mst)
                    else:
                        mbf = mstp.tile([128, MQC, 128], BF16, tag="mbf",
                                        bufs=4, name=f"mbf_{kt}_{qtr}")
                        nc.vector.tensor_scalar(
                            out=mbf, in0=mst, scalar1=-1.0, scalar2=1.0,
                            op0=ALU.mult, op1=ALU.add)
                        nc.sync.dma_start(out=dst, in_=mbf)
                    out_state[("mscr", kt, qtr)] = mscr
                return fn

            def m_transpose(kt, qtr):
                def fn():
                    mscr = out_state.pop(("mscr", kt, qtr))
                    nc.sync.dma_start(
                        out=maskT[kt][:, qtr * QBLK:(qtr + 1) * QBLK],
                        in_=mscr, transpose=True)
                return fn

            for qtr in range(NQ):
                for kt in range(NKJ):
                    Q = qtr * NKJ + kt
                    C = Q + SKEW
                    at(C - 10, m_load(kt, qtr))
                    at(C - 7, m_stage(kt, qtr))
                    at(C - 5, m_transpose(kt, qtr))

            # ---------------- output phase (spread) ----------------
            def out_copy(g, i):
                def fn():
                    h = 2 * (g // NQ) + i
                    qb = g % NQ
                    po = ps_o[g % 2][i]
                    ot = outp.tile([DV1, QBLK], F32, tag="ot",
                                   name=f"ot_{h}_{qb}")
                    nc.vector.tensor_copy(ot, po[0:DV1, :])
                    out_state[("ot", g, i)] = ot
                return fn

            def out_tr(g, i, c0):
                def fn():
                    h = 2 * (g // NQ) + i
                    qb = g % NQ
                    ot = out_state[("ot", g, i)]
                    key = ("pnat", g, i)
                    if key not in out_state:
                        out_state[key] = tpsum.tile(
                            [128, OC, DV1], F32, tag="t",
                            name=f"pnat_{h}_{qb}")
                    pn = out_state[key]
                    for c in (c0, c0 + 1):
                        nc.tensor.transpose(
                            pn[:, c, :], ot[:, c * 128:(c + 1) * 128],
                            ident_f)
                    if c0 + 2 == OC:
                        out_state.pop(("ot", g, i))
                return fn

            def out_norm(g, i):
                def fn():
                    h = 2 * (g // NQ) + i
                    qb = g % NQ
                    pn = out_state.pop(("pnat", g, i))
                    rec = small.tile([128, OC], F32, tag="rec",
                                     name=f"rec_{h}_{qb}")
                    nc.vector.reciprocal(rec, pn[:, :, DK])
                    osb = outp.tile([128, OC, DK], F32, tag="osb",
                                    name=f"osb_{h}_{qb}")
                    rb = bass.AP(tensor=rec.tensor, offset=rec.offset,
                                 ap=[rec.ap[0], rec.ap[-1], [0, DK]])
                    nc.vector.tensor_mul(osb, pn[:, :, 0:DK], rb)
                    out_state[("osb", g, i)] = osb
                return fn

            def out_store(g, i):
                def fn():
                    h = 2 * (g // NQ) + i
                    qb = g % NQ
                    osb = out_state.pop(("osb", g, i))
                    nc.sync.dma_start(
                        out=o_d[h, qb * QBLK:(qb + 1) * QBLK, :].rearrange(
                            "(c p) d -> p c d", p=128),
                        in_=osb)
                return fn

            for g in range(NP * NQ):
                E = g * NKJ + NKJ - 1 + SKEW
                at(E + 1, out_copy(g, 0))
                at(E + 2, out_copy(g, 1))
                at(E + 3, out_tr(g, 0, 0))
                at(E + 4, out_tr(g, 0, 2))
                at(E + 5, out_norm(g, 0))   # frees tpsum slot for h1
                at(E + 6, out_tr(g, 1, 0))
                at(E + 6, out_store(g, 0))
                at(E + 7, out_tr(g, 1, 2))
                at(E + 8, out_norm(g, 1))
                at(E + 9, out_store(g, 1))

            # ---------------- main schedule loop ----------------
            def front(s):
                hp, qb, kj = s // SPP, (s // NKJ) % NQ, s % NKJ
                q0 = qb * QBLK
                kT = kT2[hp % 2]
                qT = qT2[hp % 2]
                ps_s = spsum.tile([128, 2 * QBLK], F32, tag="s",
                                  name=f"ps_s_{s}")
                for i in (0, 1):
                    nc.tensor.matmul(
                        ps_s[:, i * QBLK:(i + 1) * QBLK],
                        kT[64 * i:64 * i + DK, kj * 128:(kj + 1) * 128],
                        qT[64 * i:64 * i + DK, q0:q0 + QBLK],
                        start=True, stop=True)
                e_t = ep.tile([128, 2 * QBLK], BF16, tag="e", name=f"e_{s}")
                nc.scalar.activation(e_t, ps_s, AF.Exp, scale=scale)
                e_ts[s] = e_t

            def back(s):
                hp, qb, kj = s // SPP, (s // NKJ) % NQ, s % NKJ
                q0 = qb * QBLK
                e_t = e_ts.pop(s)
                msl = maskT[kj][:, q0:q0 + QBLK]
                mdup = bass.AP(tensor=msl.tensor, offset=msl.offset,
                               ap=[msl.ap[0], [0, 2], msl.ap[-1]])
                if MASK_VIA_CAST_DMA:
                    # e *= (m == 0), one DVE op on the raw 0/1 mask
                    nc.vector.scalar_tensor_tensor(
                        out=e_t, in0=mdup, scalar=0.0, in1=e_t,
                        op0=ALU.is_equal, op1=ALU.mult)
                else:
                    nc.vector.tensor_mul(e_t, e_t, mdup)
                g = s // NKJ
                if kj == 0:
                    ps_o[g % 2] = [
                        opsum.tile([PVM, QBLK], F32, tag="o",
                                   name=f"ps_o_{g}_{i}")
                        for i in (0, 1)
                    ]
                for i in (0, 1):
                    nc.tensor.matmul(
                        ps_o[g % 2][i],
                        v1s[hp % 2][i][:, kj, :],
                        e_t[:, i * QBLK:(i + 1) * QBLK],
                        start=(kj == 0), stop=(kj == NKJ - 1),
                        skip_group_check=True)

            last_slot = max(sched.keys())
            for t in range(max(TOT + SKEW, last_slot + 1)):
                if t == 0:
                    for fn in sched.get(t, ()):
                        fn()
                if t < TOT:
                    front(t)
                if SKEW <= t < TOT + SKEW:
                    back(t - SKEW)
                if t > 0:
                    for fn in sched.get(t, ()):
                        fn()

    nc.compile()
    return nc


_NC_CACHE: dict = {}


def _get_nc(nheads, S, DK, scale):
    key = (nheads, S, DK, scale)
    if key not in _NC_CACHE:
        _NC_CACHE[key] = build_attention_nc(nheads, S, DK, scale)
    return _NC_CACHE[key]


def kernel(queries, keys, values, d_k, mask):
    from concourse.bass_utils import run_bass_kernel_spmd

    B, H, S, DK = queries.shape
    BH = B * H
    assert BH % N_CORES == 0
    hpc = BH // N_CORES
    scale = 1.0 / math.sqrt(float(d_k))

    nc = _get_nc(hpc, S, DK, scale)

    qf = np.ascontiguousarray(queries.reshape(BH, S, DK)).astype(np.float32)
    kf = np.ascontiguousarray(keys.reshape(BH, S, DK)).astype(np.float32)
    vf = np.ascontiguousarray(values.reshape(BH, S, DK)).astype(np.float32)
    mf = np.ascontiguousarray(mask.reshape(S, S)).astype(np.int32)

    in_maps = [
        {
            "queries": qf[c * hpc : (c + 1) * hpc],
            "keys": kf[c * hpc : (c + 1) * hpc],
            "values": vf[c * hpc : (c + 1) * hpc],
            "mask": mf,
        }
        for c in range(N_CORES)
    ]
    res = run_bass_kernel_spmd(nc, in_maps, core_ids=list(range(N_CORES)))
    out = np.concatenate([r["out"] for r in res.results], axis=0)
    return out.reshape(B, H, S, DK).astype(queries.dtype)


# revision 8
# speedup vs baseline: 1.2165x; 1.2165x over previous
"""Masked dot-product attention on 8 Trainium2 NeuronCores.

Strategy (per core): head-parallel sharding. B*H = 64 (batch, head) pairs are
split 8 per core; each core runs the full attention for its heads, two heads
("a pair") at a time so the K=64 QK^T matmuls co-run on PE row-group halves.

Per-pair pipeline (S=2048, DK=64), in "S-transposed" layout so the PV matmul
needs no transpose of the huge exp matrix:
  S_T[kj, qi] = K @ Q^T        (PE, bf16, psum [128 kj, 2*512 qi] strips,
                                two heads row-group packed -> co-run)
  E_T = exp(S_T / sqrt(dk))    (ACT, PSUM -> SBUF bf16; no max-shift: logits
                                are ~N(0,1) so exp can't overflow)
  E_T *= maskT (0/1 bf16)      (DVE, 2x mode, in-place)
  O_T[dv, qi] += V1[kj]^T E_T  (PE accumulate over kj; V1 has a ones column
                                so row dv=64 accumulates the softmax denom)
  O = (O_T^T)[:, :64] * recip(O_T^T[:, 64])   (PE transpose + DVE)

ACT's exp (1 elem/lane/cycle, ScalarE-only) is the hard engine floor
(~260us); everything else is scheduled to hide under it:
  - Q/K transposes and the mask transpose go through DRAM scratch + the DMA
    XBAR transpose; the sync queue carries ONLY transposes (they execute on
    the issuing engine), all plain loads/stores issue from the scalar HWDGE
    queue (issue is free, execution is on the DMA engines)
  - the int32 [S, S] mask streams in half-strips in exactly consumption
    order, converted int32->bf16 by a gpsimd SWDGE casting DMA (no DVE/ACT
    cost), flipped to (1-m) in place by a cheap DVE 4x tensor_scalar
  - emission is software-pipelined with skew 2: the QK/EXP stream runs two
    steps ahead of mask-mult/PV, so a late mask half can't stall ACT
    through the in-order PE queue
  - output normalization is spread over the following block's steps
"""

import math
from collections import defaultdict

import numpy as np

import concourse.bass as bass
import concourse.mybir as mybir
import concourse.tile as tile
from concourse import bacc
from concourse.masks import make_identity

F32 = mybir.dt.float32
BF16 = mybir.dt.bfloat16
I32 = mybir.dt.int32
AF = mybir.ActivationFunctionType
ALU = mybir.AluOpType

N_CORES = 8


def build_attention_nc(nheads: int, S: int, DK: int, scale: float) -> bass.Bass:
    nc = bacc.Bacc("TRN2", target_bir_lowering=False, debug=False,
                   num_devices=N_CORES)

    q_d = nc.dram_tensor("queries", [nheads, S, DK], F32, kind="ExternalInput")
    k_d = nc.dram_tensor("keys", [nheads, S, DK], F32, kind="ExternalInput")
    v_d = nc.dram_tensor("values", [nheads, S, DK], F32, kind="ExternalInput")
    m_d = nc.dram_tensor("mask", [S, S], I32, kind="ExternalInput")
    o_d = nc.dram_tensor("out", [nheads, S, DK], F32, kind="ExternalOutput")

    DV1 = DK + 1          # V plus a ones column for softmax denominators
    NKJ = S // 128        # kj strips
    QBLK = 512
    NQ = S // QBLK        # qi blocks per head
    OC = QBLK // 128      # 128-row output chunks per block
    CH = S // 128         # (p c) staging chunks
    HBLK = 2 * QBLK       # mask half-strip qi span
    MHC = HBLK // 128     # mask staging chunks per half
    NH = S // HBLK        # mask halves per strip (2)
    NP = nheads // 2      # head pairs
    SPP = NQ * NKJ        # steps per pair (64)
    TOT = NP * SPP        # total steps (256)
    SKEW = 2              # QK/EXP emission runs this many steps ahead

    assert nheads % 2 == 0

    # emission priorities within a slot (lower = earlier)
    P_LOAD, P_CAST, P_SCR, P_MT, P_QKT, P_FLIP, P_FRONT, P_BACK, P_OUT = (
        range(9))

    with tile.TileContext(nc) as tc:
        with (
            tc.tile_pool(name="consts", bufs=1) as consts,
            tc.tile_pool(name="maskT", bufs=1) as maskpool,
            tc.tile_pool(name="mst", bufs=4) as mstp,
            tc.tile_pool(name="stage", bufs=5) as stage,
            tc.tile_pool(name="qkT", bufs=2) as qkt,
            tc.tile_pool(name="vp", bufs=2) as vp,
            tc.tile_pool(name="ep", bufs=8) as ep,
            tc.tile_pool(name="outp", bufs=2) as outp,
            tc.tile_pool(name="small", bufs=2) as small,
            tc.tile_pool(name="spsum", bufs=2, space="PSUM") as spsum,
            tc.tile_pool(name="opsum", bufs=3, space="PSUM") as opsum,
            tc.tile_pool(name="tpsum", bufs=1, space="PSUM") as tpsum,
            tc.tile_pool(name="dram_scr", bufs=2, space="DRAM") as dram_scr,
        ):
            ident_f = consts.tile([DV1, DV1], F32)
            make_identity(nc, ident_f)

            maskT = [
                maskpool.tile([128, S], BF16, tag=f"maskT{kt}",
                              name=f"maskT_{kt}")
                for kt in range(NKJ)
            ]

            qT2 = [None, None]
            kT2 = [None, None]
            v1s = [[None, None], [None, None]]
            ps_o = {}
            e_ts = {}
            st = {}

            events = []   # (slot, prio, seq, fn)
            seq_ctr = [0]

            def at(slot, prio, fn):
                events.append((max(0, slot), prio, seq_ctr[0], fn))
                seq_ctr[0] += 1

            # ---------------- per-pair q/k/v prefetch ----------------
            def qk_load(hp, name, src, i):
                def fn():
                    nat = stage.tile([128, CH, DK], F32, tag="nat",
                                     name=f"nat_{name}_{hp}_{i}")
                    nc.scalar.dma_start(
                        out=nat,
                        in_=src[2 * hp + i].rearrange("(p c) d -> p c d",
                                                      p=128))
                    st[("nat", hp, name, i)] = nat
                return fn

            def qk_cast(hp, name, i):
                def fn():
                    nat = st.pop(("nat", hp, name, i))
                    key = ("natb", hp, name)
                    if key not in st:
                        st[key] = stage.tile(
                            [128, CH, 2 * DK], BF16, tag="natb", bufs=2,
                            name=f"natb_{name}_{hp}")
                    nc.vector.tensor_copy(
                        st[key][:, :, i * DK:(i + 1) * DK], nat)
                return fn

            def qk_scr(hp, name):
                def fn():
                    natb = st.pop(("natb", hp, name))
                    scr = dram_scr.tile([S, 2 * DK], BF16, tag=f"{name}scr",
                                        name=f"scr_{name}_{hp}")
                    nc.scalar.dma_start(
                        out=scr.rearrange("(p c) e -> p c e", p=128),
                        in_=natb)
                    st[("scr", hp, name)] = scr
                return fn

            def qk_transpose(hp, name):
                def fn():
                    scr = st.pop(("scr", hp, name))
                    tT = qkt.tile([128, S], BF16, tag=f"{name}T",
                                  name=f"{name}T_{hp}")
                    nc.sync.dma_start(out=tT, in_=scr, transpose=True)
                    (qT2 if name == "q" else kT2)[hp % 2] = tT
                return fn

            def v_load(hp, i):
                def fn():
                    vnat = stage.tile([128, CH, DK], F32, tag="vnat", bufs=3,
                                      name=f"vnat_{hp}_{i}")
                    nc.scalar.dma_start(
                        out=vnat,
                        in_=v_d[2 * hp + i].rearrange("(c p) d -> p c d",
                                                      p=128))
                    st[("vnat", hp, i)] = vnat
                return fn

            def v_build(hp, i):
                def fn():
                    vnat = st.pop(("vnat", hp, i))
                    v1 = vp.tile([128, CH, DV1], BF16, tag=f"v1_{i}",
                                 name=f"v1_{2 * hp + i}")
                    nc.vector.tensor_copy(v1[:, :, 0:DK], vnat)
                    nc.gpsimd.memset(v1[:, :, DK:DV1], 1.0)
                    v1s[hp % 2][i] = v1
                return fn

            for hp in range(NP):
                base = (hp - 1) * SPP + 8   # inside previous pair's steps
                for i in (0, 1):
                    at(base + i, P_LOAD, qk_load(hp, "k", k_d, i))
                    at(base + 2 + i, P_LOAD, qk_load(hp, "q", q_d, i))
                    at(base + 6 + i, P_CAST, qk_cast(hp, "k", i))
                    at(base + 8 + i, P_CAST, qk_cast(hp, "q", i))
                at(base + 10, P_SCR, qk_scr(hp, "k"))
                at(base + 11, P_SCR, qk_scr(hp, "q"))
                at(base + 32, P_QKT, qk_transpose(hp, "k"))
                at(base + 36, P_QKT, qk_transpose(hp, "q"))
                at(base + 14, P_LOAD, v_load(hp, 0))
                at(base + 16, P_LOAD, v_load(hp, 1))
                at(base + 40, P_CAST, v_build(hp, 0))
                at(base + 42, P_CAST, v_build(hp, 1))

            # ---------------- mask half-strip pipeline ----------------
            # half H = h*NKJ + kt covers qi [h*1024, (h+1)*1024) of strip kt;
            # first consumed by back-step s_c = h*2*NKJ + kt (pair 0).
            def m_load(kt, h):
                def fn():
                    mst = mstp.tile([128, MHC, 128], I32, tag="mraw",
                                    name=f"mraw_{kt}_{h}")
                    src = m_d[h * HBLK:(h + 1) * HBLK,
                              kt * 128:(kt + 1) * 128].rearrange(
                                  "(p c) j -> p c j", p=128)
                    nc.scalar.dma_start(out=mst, in_=src)
                    st[("mst", kt, h)] = mst
                return fn

            def m_stage(kt, h):
                def fn():
                    mst = st.pop(("mst", kt, h))
                    mscr = dram_scr.tile([HBLK, 128], BF16, tag="mscr",
                                         bufs=4, name=f"mscr_{kt}_{h}")
                    # SWDGE casting DMA: int32 {0,1} -> bf16 {0.0,1.0}
                    nc.gpsimd.dma_start(
                        out=mscr.rearrange("(p c) j -> p c j", p=128),
                        in_=mst)
                    st[("mscr", kt, h)] = mscr
                return fn

            def m_transpose(kt, h):
                def fn():
                    mscr = st.pop(("mscr", kt, h))
                    nc.sync.dma_start(
                        out=maskT[kt][:, h * HBLK:(h + 1) * HBLK],
                        in_=mscr, transpose=True)
                return fn

            def m_flip(kt, h):
                def fn():
                    # in-place keep = 1 - m  (bf16 single-src, 4x mode)
                    sl = maskT[kt][:, h * HBLK:(h + 1) * HBLK]
                    nc.vector.tensor_scalar(
                        out=sl, in0=sl, scalar1=-1.0, scalar2=1.0,
                        op0=ALU.mult, op1=ALU.add)
                return fn

            for h in range(NH):
                for kt in range(NKJ):
                    s_c = h * 2 * NKJ + kt
                    at(s_c - 12, P_LOAD, m_load(kt, h))
                    at(s_c - 8, P_SCR, m_stage(kt, h))
                    at(s_c - 5, P_MT, m_transpose(kt, h))
                    at(s_c - 2, P_FLIP, m_flip(kt, h))

            # ---------------- output phase (spread) ----------------
            def out_copy(g, i):
                def fn():
                    h = 2 * (g // NQ) + i
                    qb = g % NQ
                    ot = outp.tile([DV1, QBLK], F32, tag="ot",
                                   name=f"ot_{h}_{qb}")
                    nc.vector.tensor_copy(ot, ps_o[g % 3][i])
                    st[("ot", g, i)] = ot
                return fn

            def out_tr(g, i, c0):
                def fn():
                    h = 2 * (g // NQ) + i
                    qb = g % NQ
                    ot = st[("ot", g, i)]
                    key = ("pnat", g, i)
                    if key not in st:
                        st[key] = tpsum.tile(
                            [128, OC, DV1], F32, tag="t",
                            name=f"pnat_{h}_{qb}")
                    pn = st[key]
                    for c in (c0, c0 + 1):
                        nc.tensor.transpose(
                            pn[:, c, :], ot[:, c * 128:(c + 1) * 128],
                            ident_f)
                    if c0 + 2 == OC:
                        st.pop(("ot", g, i))
                return fn

            def out_norm(g, i):
                def fn():
                    h = 2 * (g // NQ) + i
                    qb = g % NQ
                    pn = st.pop(("pnat", g, i))
                    rec = small.tile([128, OC], F32, tag="rec",
                                     name=f"rec_{h}_{qb}")
                    nc.vector.reciprocal(rec, pn[:, :, DK])
                    osb = outp.tile([128, OC, DK], F32, tag="osb",
                                    name=f"osb_{h}_{qb}")
                    rb = bass.AP(tensor=rec.tensor, offset=rec.offset,
                                 ap=[rec.ap[0], rec.ap[-1], [0, DK]])
                    nc.vector.tensor_mul(osb, pn[:, :, 0:DK], rb)
                    st[("osb", g, i)] = osb
                return fn

            def out_store(g, i):
                def fn():
                    h = 2 * (g // NQ) + i
                    qb = g % NQ
                    osb = st.pop(("osb", g, i))
                    nc.scalar.dma_start(
                        out=o_d[h, qb * QBLK:(qb + 1) * QBLK, :].rearrange(
                            "(c p) d -> p c d", p=128),
                        in_=osb)
                return fn

            for g in range(NP * NQ):
                E = g * NKJ + NKJ - 1 + SKEW
                at(E + 1, P_OUT, out_copy(g, 0))
                at(E + 2, P_OUT, out_copy(g, 1))
                at(E + 3, P_OUT, out_tr(g, 0, 0))
                at(E + 4, P_OUT, out_tr(g, 0, 2))
                at(E + 5, P_OUT, out_norm(g, 0))   # frees tpsum slot for h1
                at(E + 6, P_OUT, out_tr(g, 1, 0))
                at(E + 6, P_OUT, out_store(g, 0))
                at(E + 7, P_OUT, out_tr(g, 1, 2))
                at(E + 8, P_OUT, out_norm(g, 1))
                at(E + 9, P_OUT, out_store(g, 1))

            # ---------------- main step work ----------------
            def front(s):
                def fn():
                    hp, qb, kj = s // SPP, (s // NKJ) % NQ, s % NKJ
                    q0 = qb * QBLK
                    kT, qT = kT2[hp % 2], qT2[hp % 2]
                    ps_s = spsum.tile([128, 2 * QBLK], F32, tag="s",
                                      name=f"ps_s_{s}")
                    for i in (0, 1):
                        nc.tensor.matmul(
                            ps_s[:, i * QBLK:(i + 1) * QBLK],
                            kT[64 * i:64 * i + DK, kj * 128:(kj + 1) * 128],
                            qT[64 * i:64 * i + DK, q0:q0 + QBLK],
                            start=True, stop=True)
                    e_t = ep.tile([128, 2 * QBLK], BF16, tag="e",
                                  name=f"e_{s}")
                    nc.scalar.activation(e_t, ps_s, AF.Exp, scale=scale)
                    e_ts[s] = e_t
                return fn

            def back(s):
                def fn():
                    hp, qb, kj = s // SPP, (s // NKJ) % NQ, s % NKJ
                    q0 = qb * QBLK
                    e_t = e_ts.pop(s)
                    msl = maskT[kj][:, q0:q0 + QBLK]
                    mdup = bass.AP(tensor=msl.tensor, offset=msl.offset,
                                   ap=[msl.ap[0], [0, 2], msl.ap[-1]])
                    nc.vector.tensor_mul(e_t, e_t, mdup)
                    g = s // NKJ
                    if kj == 0:
                        ps_o[g % 3] = [
                            opsum.tile([DV1, QBLK], F32, tag="o",
                                       name=f"ps_o_{g}_{i}")
                            for i in (0, 1)
                        ]
                    for i in (0, 1):
                        nc.tensor.matmul(
                            ps_o[g % 3][i],
                            v1s[hp % 2][i][:, kj, :],
                            e_t[:, i * QBLK:(i + 1) * QBLK],
                            start=(kj == 0), stop=(kj == NKJ - 1),
                            skip_group_check=True)
                return fn

            for s in range(TOT):
                at(s, P_FRONT, front(s))
                at(s + SKEW, P_BACK, back(s))

            events.sort(key=lambda e: (e[0], e[1], e[2]))
            for _, _, _, fn in events:
                fn()

    nc.compile()
    return nc


_NC_CACHE: dict = {}


def _get_nc(nheads, S, DK, scale):
    key = (nheads, S, DK, scale)
    if key not in _NC_CACHE:
        _NC_CACHE[key] = build_attention_nc(nheads, S, DK, scale)
    return _NC_CACHE[key]


def kernel(queries, keys, values, d_k, mask):
    from concourse.bass_utils import run_bass_kernel_spmd

    B, H, S, DK = queries.shape
    BH = B * H
    assert BH % N_CORES == 0
    hpc = BH // N_CORES
    scale = 1.0 / math.sqrt(float(d_k))

    nc = _get_nc(hpc, S, DK, scale)

    qf = np.ascontiguousarray(queries.reshape(BH, S, DK)).astype(np.float32)
    kf = np.ascontiguousarray(keys.reshape(BH, S, DK)).astype(np.float32)
    vf = np.ascontiguousarray(values.reshape(BH, S, DK)).astype(np.float32)
    mf = np.ascontiguousarray(mask.reshape(S, S)).astype(np.int32)

    in_maps = [
        {
            "queries": qf[c * hpc : (c + 1) * hpc],
            "keys": kf[c * hpc : (c + 1) * hpc],
            "values": vf[c * hpc : (c + 1) * hpc],
            "mask": mf,
        }
        for c in range(N_CORES)
    ]
    res = run_bass_kernel_spmd(nc, in_maps, core_ids=list(range(N_CORES)))
    out = np.concatenate([r["out"] for r in res.results], axis=0)
    return out.reshape(B, H, S, DK).astype(queries.dtype)


# revision 14
# speedup vs baseline: 1.2305x; 1.0115x over previous
"""Masked dot-product attention on 8 Trainium2 NeuronCores.

Strategy (per core): head-parallel sharding. B*H = 64 (batch, head) pairs are
split 8 per core; each core runs the full attention for its heads, two heads
("a pair") at a time so the K=64 QK^T matmuls co-run on PE row-group halves.

Per-pair pipeline (S=2048, DK=64), in "S-transposed" layout so the PV matmul
needs no transpose of the huge exp matrix:
  S_T[kj, qi] = K @ Q^T        (PE, bf16, psum [128 kj, 2*512 qi] strips,
                                two heads row-group packed -> co-run)
  E_T = exp(S_T / sqrt(dk))    (ACT, PSUM -> SBUF bf16; no max-shift: logits
                                are ~N(0,1) so exp can't overflow)
  E_T *= maskT (0/1 bf16)      (DVE, 2x mode, in-place)
  O_T[dv, qi] += V1[kj]^T E_T  (PE accumulate over kj; V1 has a ones column
                                so row dv=64 accumulates the softmax denom)
  O = (O_T^T)[:, :64] * recip(O_T^T[:, 64])   (PE transpose + DVE)

ACT's exp (1 elem/lane/cycle, ScalarE-only) is the hard engine floor
(~260us); everything else is scheduled to hide under it:
  - Q/K transposes and the mask transpose go through DRAM scratch + the DMA
    XBAR transpose; the sync queue carries ONLY transposes (they execute on
    the issuing engine), all plain loads/stores issue from the scalar HWDGE
    queue (issue is free, execution is on the DMA engines)
  - the int32 [S, S] mask streams in half-strips in exactly consumption
    order, converted int32->bf16 by a gpsimd SWDGE casting DMA (no DVE/ACT
    cost), flipped to (1-m) in place by a cheap DVE 4x tensor_scalar
  - emission is software-pipelined with skew 2: the QK/EXP stream runs two
    steps ahead of mask-mult/PV, so a late mask half can't stall ACT
    through the in-order PE queue
  - output normalization is spread over the following block's steps
"""

import math
from collections import defaultdict

import numpy as np

import concourse.bass as bass
import concourse.mybir as mybir
import concourse.tile as tile
from concourse import bacc
from concourse.masks import make_identity

F32 = mybir.dt.float32
BF16 = mybir.dt.bfloat16
I32 = mybir.dt.int32
AF = mybir.ActivationFunctionType
ALU = mybir.AluOpType

N_CORES = 8


def build_attention_nc(nheads: int, S: int, DK: int, scale: float) -> bass.Bass:
    nc = bacc.Bacc("TRN2", target_bir_lowering=False, debug=False,
                   num_devices=N_CORES)

    q_d = nc.dram_tensor("queries", [nheads, S, DK], F32, kind="ExternalInput")
    k_d = nc.dram_tensor("keys", [nheads, S, DK], F32, kind="ExternalInput")
    v_d = nc.dram_tensor("values", [nheads, S, DK], F32, kind="ExternalInput")
    m_d = nc.dram_tensor("mask", [S, S], I32, kind="ExternalInput")
    o_d = nc.dram_tensor("out", [nheads, S, DK], F32, kind="ExternalOutput")

    DV1 = DK + 1          # V plus a ones column for softmax denominators
    NKJ = S // 128        # kj strips
    QBLK = 512
    NQ = S // QBLK        # qi blocks per head
    OC = QBLK // 128      # 128-row output chunks per block
    CH = S // 128         # (p c) staging chunks
    HBLK = 2 * QBLK       # mask half-strip qi span
    MHC = HBLK // 128     # mask staging chunks per half
    NH = S // HBLK        # mask halves per strip (2)
    NP = nheads // 2      # head pairs
    SPP = NQ * NKJ        # steps per pair (64)
    TOT = NP * SPP        # total steps (256)
    SKEW = 2              # QK/EXP emission runs this many steps ahead

    assert nheads % 2 == 0

    # emission priorities within a slot (lower = earlier)
    (P_LOAD, P_CAST, P_SCR, P_QKT, P_MSTAGE, P_MT, P_FLIP, P_FRONT, P_BACK,
     P_OUT) = range(10)

    with tile.TileContext(nc) as tc:
        with (
            tc.tile_pool(name="consts", bufs=1) as consts,
            tc.tile_pool(name="maskT", bufs=1) as maskpool,
            tc.tile_pool(name="mst", bufs=6) as mstp,
            tc.tile_pool(name="stage", bufs=5) as stage,
            tc.tile_pool(name="qkT", bufs=2) as qkt,
            tc.tile_pool(name="vp", bufs=2) as vp,
            tc.tile_pool(name="ep", bufs=8) as ep,
            tc.tile_pool(name="outp", bufs=2) as outp,
            tc.tile_pool(name="small", bufs=2) as small,
            tc.tile_pool(name="spsum", bufs=2, space="PSUM") as spsum,
            tc.tile_pool(name="opsum", bufs=3, space="PSUM") as opsum,
            tc.tile_pool(name="tpsum", bufs=1, space="PSUM") as tpsum,
            tc.tile_pool(name="dram_scr", bufs=2, space="DRAM") as dram_scr,
        ):
            ident_f = consts.tile([DV1, DV1], F32)
            make_identity(nc, ident_f)

            maskT = [
                maskpool.tile([128, S], BF16, tag=f"maskT{kt}",
                              name=f"maskT_{kt}")
                for kt in range(NKJ)
            ]

            qT2 = [None, None]
            kT2 = [None, None]
            v1s = [[None, None], [None, None]]
            ps_o = {}
            e_ts = {}
            st = {}

            events = []   # (slot, prio, seq, fn)
            seq_ctr = [0]

            def at(slot, prio, fn):
                events.append((max(0, slot), prio, seq_ctr[0], fn))
                seq_ctr[0] += 1

            # ---------------- per-pair q/k/v prefetch ----------------
            def qk_load(hp, name, src, i):
                def fn():
                    nat = stage.tile([128, CH, DK], F32, tag="nat",
                                     name=f"nat_{name}_{hp}_{i}")
                    nc.scalar.dma_start(
                        out=nat,
                        in_=src[2 * hp + i].rearrange("(p c) d -> p c d",
                                                      p=128))
                    st[("nat", hp, name, i)] = nat
                return fn

            def qk_cast(hp, name, i):
                def fn():
                    nat = st.pop(("nat", hp, name, i))
                    key = ("natb", hp, name)
                    if key not in st:
                        st[key] = stage.tile(
                            [128, CH, 2 * DK], BF16, tag="natb", bufs=2,
                            name=f"natb_{name}_{hp}")
                    nc.vector.tensor_copy(
                        st[key][:, :, i * DK:(i + 1) * DK], nat)
                return fn

            def qk_scr(hp, name):
                def fn():
                    natb = st.pop(("natb", hp, name))
                    scr = dram_scr.tile([S, 2 * DK], BF16, tag=f"{name}scr",
                                        name=f"scr_{name}_{hp}")
                    nc.sync.dma_start(
                        out=scr.rearrange("(p c) e -> p c e", p=128),
                        in_=natb)
                    st[("scr", hp, name)] = scr
                return fn

            def qk_transpose(hp, name):
                def fn():
                    scr = st.pop(("scr", hp, name))
                    tT = qkt.tile([128, S], BF16, tag=f"{name}T",
                                  name=f"{name}T_{hp}")
                    nc.sync.dma_start(out=tT, in_=scr, transpose=True)
                    (qT2 if name == "q" else kT2)[hp % 2] = tT
                return fn

            def v_load(hp, i):
                def fn():
                    vnat = stage.tile([128, CH, DK], F32, tag="vnat", bufs=3,
                                      name=f"vnat_{hp}_{i}")
                    nc.scalar.dma_start(
                        out=vnat,
                        in_=v_d[2 * hp + i].rearrange("(c p) d -> p c d",
                                                      p=128))
                    st[("vnat", hp, i)] = vnat
                return fn

            def v_build(hp, i):
                def fn():
                    vnat = st.pop(("vnat", hp, i))
                    v1 = vp.tile([128, CH, DV1], BF16, tag=f"v1_{i}",
                                 name=f"v1_{2 * hp + i}")
                    nc.vector.tensor_copy(v1[:, :, 0:DK], vnat)
                    nc.vector.memset(v1[:, :, DK:DV1], 1.0)
                    v1s[hp % 2][i] = v1
                return fn

            for hp in range(NP):
                base = (hp - 1) * SPP + 8   # inside previous pair's steps
                for i in (0, 1):
                    at(base + i, P_LOAD, qk_load(hp, "k", k_d, i))
                    at(base + 2 + i, P_LOAD, qk_load(hp, "q", q_d, i))
                    at(base + 6 + i, P_CAST, qk_cast(hp, "k", i))
                    at(base + 8 + i, P_CAST, qk_cast(hp, "q", i))
                at(base + 10, P_SCR, qk_scr(hp, "k"))
                at(base + 11, P_SCR, qk_scr(hp, "q"))
                at(base + 14, P_QKT, qk_transpose(hp, "k"))
                at(base + 16, P_QKT, qk_transpose(hp, "q"))
                at(base + 14, P_LOAD, v_load(hp, 0))
                at(base + 16, P_LOAD, v_load(hp, 1))
                at(base + 40, P_CAST, v_build(hp, 0))
                at(base + 42, P_CAST, v_build(hp, 1))

            # ---------------- mask half-strip pipeline ----------------
            # half H = h*NKJ + kt covers qi [h*1024, (h+1)*1024) of strip kt;
            # first consumed by back-step s_c = h*2*NKJ + kt (pair 0).
            def m_load(kt, h):
                def fn():
                    mst = mstp.tile([128, MHC, 128], I32, tag="mraw",
                                    name=f"mraw_{kt}_{h}")
                    src = m_d[h * HBLK:(h + 1) * HBLK,
                              kt * 128:(kt + 1) * 128].rearrange(
                                  "(p c) j -> p c j", p=128)
                    nc.scalar.dma_start(out=mst, in_=src)
                    st[("mst", kt, h)] = mst
                return fn

            def m_stage(kt, h):
                def fn():
                    mst = st.pop(("mst", kt, h))
                    mscr = dram_scr.tile([HBLK, 128], BF16, tag="mscr",
                                         bufs=4, name=f"mscr_{kt}_{h}")
                    # SWDGE casting DMA: int32 {0,1} -> bf16 {0.0,1.0}
                    nc.gpsimd.dma_start(
                        out=mscr.rearrange("(p c) j -> p c j", p=128),
                        in_=mst)
                    st[("mscr", kt, h)] = mscr
                return fn

            def m_transpose(kt, h):
                def fn():
                    mscr = st.pop(("mscr", kt, h))
                    nc.sync.dma_start(
                        out=maskT[kt][:, h * HBLK:(h + 1) * HBLK],
                        in_=mscr, transpose=True)
                return fn

            def m_flip(kt, h):
                def fn():
                    # in-place keep = 1 - m  (bf16 single-src, 4x mode)
                    sl = maskT[kt][:, h * HBLK:(h + 1) * HBLK]
                    nc.vector.tensor_scalar(
                        out=sl, in0=sl, scalar1=-1.0, scalar2=1.0,
                        op0=ALU.mult, op1=ALU.add)
                return fn

            for h in range(NH):
                for kt in range(NKJ):
                    s_c = h * 2 * NKJ + kt
                    at(s_c - 12, P_LOAD, m_load(kt, h))
                    at(s_c - 8, P_MSTAGE, m_stage(kt, h))
                    at(s_c - 5, P_MT, m_transpose(kt, h))
                    at(s_c - 2, P_FLIP, m_flip(kt, h))

            # ---------------- output phase (spread) ----------------
            def out_copy(g, i):
                def fn():
                    h = 2 * (g // NQ) + i
                    qb = g % NQ
                    ot = outp.tile([DV1, QBLK], F32, tag="ot",
                                   name=f"ot_{h}_{qb}")
                    nc.vector.tensor_copy(ot, ps_o[g % 3][i])
                    st[("ot", g, i)] = ot
                return fn

            def out_tr(g, i, c0):
                def fn():
                    h = 2 * (g // NQ) + i
                    qb = g % NQ
                    ot = st[("ot", g, i)]
                    key = ("pnat", g, i)
                    if key not in st:
                        st[key] = tpsum.tile(
                            [128, OC, DV1], F32, tag="t",
                            name=f"pnat_{h}_{qb}")
                    pn = st[key]
                    for c in (c0, c0 + 1):
                        nc.tensor.transpose(
                            pn[:, c, :], ot[:, c * 128:(c + 1) * 128],
                            ident_f)
                    if c0 + 2 == OC:
                        st.pop(("ot", g, i))
                return fn

            def out_norm(g, i):
                def fn():
                    h = 2 * (g // NQ) + i
                    qb = g % NQ
                    pn = st.pop(("pnat", g, i))
                    rec = small.tile([128, OC], F32, tag="rec",
                                     name=f"rec_{h}_{qb}")
                    nc.vector.reciprocal(rec, pn[:, :, DK])
                    osb = outp.tile([128, OC, DK], F32, tag="osb",
                                    name=f"osb_{h}_{qb}")
                    rb = bass.AP(tensor=rec.tensor, offset=rec.offset,
                                 ap=[rec.ap[0], rec.ap[-1], [0, DK]])
                    nc.vector.tensor_mul(osb, pn[:, :, 0:DK], rb)
                    st[("osb", g, i)] = osb
                return fn

            def out_store(g, i):
                def fn():
                    h = 2 * (g // NQ) + i
                    qb = g % NQ
                    osb = st.pop(("osb", g, i))
                    nc.scalar.dma_start(
                        out=o_d[h, qb * QBLK:(qb + 1) * QBLK, :].rearrange(
                            "(c p) d -> p c d", p=128),
                        in_=osb)
                return fn

            for g in range(NP * NQ):
                E = g * NKJ + NKJ - 1 + SKEW
                at(E + 1, P_OUT, out_copy(g, 0))
                at(E + 2, P_OUT, out_copy(g, 1))
                at(E + 3, P_OUT, out_tr(g, 0, 0))
                at(E + 4, P_OUT, out_tr(g, 0, 2))
                at(E + 5, P_OUT, out_norm(g, 0))   # frees tpsum slot for h1
                at(E + 6, P_OUT, out_tr(g, 1, 0))
                at(E + 7, P_OUT, out_tr(g, 1, 2))
                at(E + 8, P_OUT, out_norm(g, 1))
                # stores late so they never semaphore-block the scalar queue
                at(E + 12, P_OUT, out_store(g, 0))
                at(E + 13, P_OUT, out_store(g, 1))

            # ---------------- main step work ----------------
            def front(s):
                def fn():
                    hp, qb, kj = s // SPP, (s // NKJ) % NQ, s % NKJ
                    q0 = qb * QBLK
                    kT, qT = kT2[hp % 2], qT2[hp % 2]
                    ps_s = spsum.tile([128, 2 * QBLK], F32, tag="s",
                                      name=f"ps_s_{s}")
                    for i in (0, 1):
                        nc.tensor.matmul(
                            ps_s[:, i * QBLK:(i + 1) * QBLK],
                            kT[64 * i:64 * i + DK, kj * 128:(kj + 1) * 128],
                            qT[64 * i:64 * i + DK, q0:q0 + QBLK],
                            start=True, stop=True)
                    e_t = ep.tile([128, 2 * QBLK], BF16, tag="e",
                                  name=f"e_{s}")
                    nc.scalar.activation(e_t, ps_s, AF.Exp, scale=scale)
                    e_ts[s] = e_t
                return fn

            def back(s):
                def fn():
                    hp, qb, kj = s // SPP, (s // NKJ) % NQ, s % NKJ
                    q0 = qb * QBLK
                    e_t = e_ts.pop(s)
                    msl = maskT[kj][:, q0:q0 + QBLK]
                    mdup = bass.AP(tensor=msl.tensor, offset=msl.offset,
                                   ap=[msl.ap[0], [0, 2], msl.ap[-1]])
                    nc.vector.tensor_mul(e_t, e_t, mdup)
                    g = s // NKJ
                    if kj == 0:
                        ps_o[g % 3] = [
                            opsum.tile([DV1, QBLK], F32, tag="o",
                                       name=f"ps_o_{g}_{i}")
                            for i in (0, 1)
                        ]
                    for i in (0, 1):
                        nc.tensor.matmul(
                            ps_o[g % 3][i],
                            v1s[hp % 2][i][:, kj, :],
                            e_t[:, i * QBLK:(i + 1) * QBLK],
                            start=(kj == 0), stop=(kj == NKJ - 1),
                            skip_group_check=True)
                return fn

            for s in range(TOT):
                at(s, P_FRONT, front(s))
                at(s + SKEW, P_BACK, back(s))

            events.sort(key=lambda e: (e[0], e[1], e[2]))
            for _, _, _, fn in events:
                fn()

    nc.compile()
    return nc


_NC_CACHE: dict = {}


def _get_nc(nheads, S, DK, scale):
    key = (nheads, S, DK, scale)
    if key not in _NC_CACHE:
        _NC_CACHE[key] = build_attention_nc(nheads, S, DK, scale)
    return _NC_CACHE[key]


def kernel(queries, keys, values, d_k, mask):
    from concourse.bass_utils import run_bass_kernel_spmd

    B, H, S, DK = queries.shape
    BH = B * H
    assert BH % N_CORES == 0
    hpc = BH // N_CORES
    scale = 1.0 / math.sqrt(float(d_k))

    nc = _get_nc(hpc, S, DK, scale)

    qf = np.ascontiguousarray(queries.reshape(BH, S, DK)).astype(np.float32)
    kf = np.ascontiguousarray(keys.reshape(BH, S, DK)).astype(np.float32)
    vf = np.ascontiguousarray(values.reshape(BH, S, DK)).astype(np.float32)
    mf = np.ascontiguousarray(mask.reshape(S, S)).astype(np.int32)

    in_maps = [
        {
            "queries": qf[c * hpc : (c + 1) * hpc],
            "keys": kf[c * hpc : (c + 1) * hpc],
            "values": vf[c * hpc : (c + 1) * hpc],
            "mask": mf,
        }
        for c in range(N_CORES)
    ]
    res = run_bass_kernel_spmd(nc, in_maps, core_ids=list(range(N_CORES)))
    out = np.concatenate([r["out"] for r in res.results], axis=0)
    return out.reshape(B, H, S, DK).astype(queries.dtype)
